# revision 25
# baseline (speedup 1.0000x reference)
"""Trainium2 Bass kernel for nn_AttnOnlyTransformer (batch 8, S=D=V=2048).

Sharding: data-parallel over batch — core b computes batch b end-to-end.
No collectives.

Math (per batch), with enc = one_hot(tok) + PE  [S, D]:
  encWt = W @ enc^T                          [D, S]   (stage 1)
  simsT = encWt^T @ enc^T ... = (enc W^T enc^T)^T / sqrt(D) in [k, q] layout
  eT    = exp(simsT * scale) * causal(k<=q)  [S, S]   (stage 2, unnormalized)
  out   = (eT^T @ enc_ext) row-normalized    [S, D]   (stage 3)
where enc_ext = [enc | 1 0 0 0]; the ones column accumulates the softmax
denominator Z in the same pass, and out = numerator * (1/Z).  Softmax without
max-subtraction is exact here: |sims/sqrt(D)| <= ~3, exp can't overflow.

All matmuls run in float32r (full PE rate at N>=512, ~1.5e-4 rel error).
Intermediates encWt and eT are staged through DRAM to fit SBUF.
"""

import numpy as np

import concourse.bass as bass  # noqa: F401  (engine types referenced via nc)
import concourse.mybir as mybir
import concourse.tile as tile
from concourse import bacc
from concourse.bass_utils import run_bass_kernel_spmd
from concourse.masks import make_upper_triangular

P = 128
S = 2048
D = 2048
T = S // P          # 16 tiles per dim
CH = 512            # matmul moving-dim chunk
NCH = S // CH       # 4 chunks
B = 8
f32 = mybir.dt.float32
f32r = mybir.dt.float32r
i32 = mybir.dt.int32
SCALE = 1.0 / float(np.sqrt(np.float32(D)))
NEG = -1.0e9


def _build():
    nc = bacc.Bacc(None, target_bir_lowering=False)
    tok = nc.dram_tensor("tok", [S], i32, kind="ExternalInput")
    wt = nc.dram_tensor("wt", [D, D], f32r, kind="ExternalInput")   # wt[d,e] = W[e,d]
    pet = nc.dram_tensor("pet", [D, S], f32r, kind="ExternalInput")  # PE^T
    pes = nc.dram_tensor("pes", [S, D], f32r, kind="ExternalInput")  # PE
    onescol = nc.dram_tensor("onescol", [P, 4], f32r, kind="ExternalInput")
    out = nc.dram_tensor("out", [S, D], f32, kind="ExternalOutput")

    pet3 = pet.rearrange("(dt p) s -> dt p s", p=P)
    pes3 = pes.rearrange("(st p) d -> st p d", p=P)
    tok2 = tok.rearrange("(st p) -> st p", p=P)
    out3 = out.rearrange("(qt p) d -> qt p d", p=P)

    with tile.TileContext(nc) as tc:
        with (
            tc.tile_pool(name="persist", bufs=1) as persist,
            tc.tile_pool(name="dram", bufs=1, space="DRAM") as dpool,
        ):
            encwt_d = dpool.tile([T, D, P], f32r)     # [kt][e][k_local]
            et_d = dpool.tile([T, T, P, P], f32r)     # [kt][qt][k][q]

            # --- persistent small tiles ---
            iota_col_i = persist.tile([P, 1], i32)
            nc.gpsimd.iota(iota_col_i[:], [[0, 1]], base=0, channel_multiplier=1)
            iota_col_f = persist.tile([P, 1], f32)
            nc.vector.tensor_copy(iota_col_f[:], iota_col_i[:])
            iota_free_i = persist.tile([P, S], i32)
            nc.gpsimd.iota(iota_free_i[:], [[1, S]], base=0, channel_multiplier=0)
            iota_free_f = persist.tile([P, S], f32)
            nc.vector.tensor_copy(iota_free_f[:], iota_free_i[:])
            # additive causal mask for the diagonal block, [k_local, q_local]:
            # 0 where k <= q (keep), NEG where k > q
            maskneg = persist.tile([P, P], f32)
            nc.gpsimd.memset(maskneg[:], 0.0)
            nc.gpsimd.affine_select(
                out=maskneg[:],
                in_=maskneg[:],
                pattern=[[1, P]],
                compare_op=mybir.AluOpType.is_ge,
                fill=NEG,
                base=0,
                channel_multiplier=-1,
            )
            dcols = []
            for dt in range(T):
                dc = persist.tile([P, 1], f32, tag=f"dcol{dt}")
                nc.vector.tensor_scalar_add(dc[:], iota_col_f[:], float(dt * P))
                dcols.append(dc)
            toksb_i = persist.tile([P, T], i32)
            nc.scalar.dma_start(toksb_i[:], tok2.rearrange("st p -> p st"))
            toksb_f = persist.tile([P, T], f32)
            nc.vector.tensor_copy(toksb_f[:], toksb_i[:])
            tokcols = [toksb_f[:, st:st + 1] for st in range(T)]
            tokrow_i = persist.tile([1, S], i32)
            nc.scalar.dma_start(tokrow_i[:], tok[None, :])

            # ================= phase A: encT, stage 1, stage 2 =================
            with (
                tc.tile_pool(name="tokbc", bufs=1) as tokbcp,
                tc.tile_pool(name="enct", bufs=1) as enctp,
                tc.tile_pool(name="wew", bufs=2) as wew,
                tc.tile_pool(name="stgA", bufs=4) as stgA,
                tc.tile_pool(name="psA", bufs=2, space="PSUM") as psA,
            ):
                tok_bc_i = tokbcp.tile([P, S], i32)
                nc.gpsimd.partition_broadcast(tok_bc_i[:], tokrow_i[:])
                tok_bc_f = tokbcp.tile([P, S], f32)
                nc.vector.tensor_copy(tok_bc_f[:], tok_bc_i[:])

                # stage 0: encT[dt][d_local, s] = PE^T[d, s] + (d == tok_s)
                encT = []
                for dt in range(T):
                    e = enctp.tile([P, S], f32r, tag=f"encT{dt}")
                    nc.sync.dma_start(e[:], pet3[dt])
                    nc.vector.scalar_tensor_tensor(
                        e[:],
                        tok_bc_f[:],
                        dcols[dt][:],
                        e[:].bitcast(f32),
                        mybir.AluOpType.is_equal,
                        mybir.AluOpType.add,
                    )
                    encT.append(e)

                # stage 1: encWt[e, k] = sum_d wt[d, e] * encT[d, k] -> DRAM kt-blocked
                # The first 6 psum groups are emitted dt-outer so the PE has
                # several independent accumulations to interleave while the
                # encT tiles are still streaming in from HBM.
                def s1_store(et, kc, ps):
                    o = stgA.tile([P, CH], f32r, tag="s1o")
                    nc.vector.tensor_copy(o[:], ps[:])
                    nc.sync.dma_start(
                        encwt_d[4 * kc:4 * kc + 4, et * P:(et + 1) * P, :]
                        .rearrange("kt p k -> p kt k"),
                        o[:].rearrange("p (kt k) -> p kt k", k=P),
                    )

                def load_w(et):
                    w_sb = wew.tile([P, T, P], f32r, tag="wew")
                    nc.scalar.dma_start(
                        w_sb[:],
                        wt[:, et * P:(et + 1) * P].rearrange("(dt p) e -> p dt e", p=P),
                    )
                    return w_sb

                w0 = load_w(0)
                w1 = load_w(1)
                head = [(0, kc) for kc in range(NCH)] + [(1, 0), (1, 1)]
                head_ps = {g: psA.tile([P, CH], f32, tag="ps1", bufs=6, name=f"hps{g[0]}_{g[1]}") for g in head}
                for dt in range(T):
                    for (et, kc) in head:
                        w_sb = w0 if et == 0 else w1
                        nc.tensor.matmul(
                            head_ps[(et, kc)][:],
                            w_sb[:, dt],
                            encT[dt][:, kc * CH:(kc + 1) * CH],
                            start=(dt == 0),
                            stop=(dt == T - 1),
                        )
                for (et, kc) in head:
                    s1_store(et, kc, head_ps[(et, kc)])

                for et in range(1, T):
                    w_sb = w1 if et == 1 else load_w(et)
                    for kc in range(2 if et == 1 else 0, NCH):
                        ps = psA.tile([P, CH], f32, tag="ps1", bufs=6)
                        for dt in range(T):
                            nc.tensor.matmul(
                                ps[:],
                                w_sb[:, dt],
                                encT[dt][:, kc * CH:(kc + 1) * CH],
                                start=(dt == 0),
                                stop=(dt == T - 1),
                            )
                        s1_store(et, kc, ps)

                # stage 2: eT[k, q] = exp(scale * sum_e encWt[e,k] encT[e,q]) * causal
                for kt in range(T):
                    ew = wew.tile([P, T, P], f32r, tag="wew")
                    nc.scalar.dma_start(
                        ew[:], encwt_d[kt].rearrange("(et p) k -> p et k", p=P)
                    )
                    qc0 = kt * P // CH
                    for qc in range(qc0, NCH):
                        ps = psA.tile([P, CH], f32, tag="ps2")
                        for et in range(T):
                            nc.tensor.matmul(
                                ps[:],
                                ew[:, et],
                                encT[et][:, qc * CH:(qc + 1) * CH],
                                start=(et == 0),
                                stop=(et == T - 1),
                            )
                        if qc == qc0:
                            off = kt * P - qc0 * CH
                            nc.vector.tensor_tensor(
                                ps[:, off:off + P],
                                ps[:, off:off + P],
                                maskneg[:],
                                mybir.AluOpType.add,
                            )
                        eo = stgA.tile([P, CH], f32r, tag="s2o")
                        nc.scalar.activation(
                            eo[:], ps[:], mybir.ActivationFunctionType.Exp, scale=SCALE
                        )
                        nc.sync.dma_start(
                            et_d[kt, 4 * qc:4 * qc + 4].rearrange("qt k q -> k qt q"),
                            eo[:].rearrange("k (qt q) -> k qt q", q=P),
                        )

            # ================= phase B: enc_ext, stage 3 =================
            with (
                tc.tile_pool(name="enc", bufs=1) as encp,
                tc.tile_pool(name="et3", bufs=2) as etp,
                tc.tile_pool(name="stgB", bufs=4) as stgB,
                tc.tile_pool(name="psB", bufs=2, space="PSUM") as psB,
            ):
                # enc_ext[st][s_local, 0:D] = PE[s, d] + (d == tok_s); [:, D:D+4] = 1,0,0,0
                enc = []

                def build_enc(st):
                    e = encp.tile([P, D + 4], f32r, tag=f"enc{st}", name=f"enc{st}")
                    eng = nc.sync if st % 2 == 0 else nc.scalar
                    eng.dma_start(e[:, 0:D], pes3[st])
                    nc.vector.scalar_tensor_tensor(
                        e[:, 0:D],
                        iota_free_f[:],
                        tokcols[st][:],
                        e[:, 0:D].bitcast(f32),
                        mybir.AluOpType.is_equal,
                        mybir.AluOpType.add,
                    )
                    nc.scalar.dma_start(e[:, D:D + 4], onescol[:])
                    enc.append(e)

                # stage 3: per q-tile, accumulate numerator and Z over k-tiles
                for qt in range(T):
                    build_enc(qt)
                    ets = []
                    for kt in range(qt + 1):
                        etile = etp.tile([P, P], f32r, tag=f"et{kt}", name=f"et{kt}", bufs=3)
                        nc.scalar.dma_start(etile[:], et_d[kt, qt])
                        ets.append(etile)
                    pss = []
                    for dc in range(NCH):
                        ps = psB.tile([P, CH], f32, tag="ps3d", bufs=6, name=f"ps3d{dc}")
                        for kt in range(qt + 1):
                            nc.tensor.matmul(
                                ps[:],
                                ets[kt][:],
                                enc[kt][:, dc * CH:(dc + 1) * CH],
                                start=(kt == 0),
                                stop=(kt == qt),
                            )
                        pss.append(ps)
                    zps = psB.tile([P, 4], f32, tag="ps3z")
                    for kt in range(qt + 1):
                        nc.tensor.matmul(
                            zps[:],
                            ets[kt][:],
                            enc[kt][:, D:D + 4],
                            start=(kt == 0),
                            stop=(kt == qt),
                        )
                    rz = stgB.tile([P, 1], f32, tag="rz")
                    nc.vector.reciprocal(rz[:], zps[:, 0:1])
                    for dc in range(NCH):
                        ob = stgB.tile([P, CH], f32, tag="ob")
                        nc.scalar.mul(ob[:], pss[dc][:], rz[:])
                        nc.sync.dma_start(out3[qt, :, dc * CH:(dc + 1) * CH], ob[:])

    nc.finalize()
    return nc


def _sinusoidal_pe(seq_len, d_model):
    pos = np.arange(seq_len, dtype=np.float32)[:, None]
    div = np.exp(
        np.arange(0, d_model, 2, dtype=np.float32) * (-np.log(10000.0) / d_model)
    ).astype(np.float32)
    ang = pos * div
    pe = np.zeros((seq_len, d_model), dtype=np.float32)
    pe[:, 0::2] = np.sin(ang)
    pe[:, 1::2] = np.cos(ang)
    return pe


_CACHED_NC = None


def _run(token_ids, W_bil, **spmd_kwargs):
    global _CACHED_NC
    if _CACHED_NC is None:
        _CACHED_NC = _build()
    nc = _CACHED_NC

    token_ids = np.asarray(token_ids)
    W = np.asarray(W_bil, dtype=np.float32)
    assert token_ids.shape == (B, S) and W.shape == (D, D)

    pe = _sinusoidal_pe(S, D)
    wt = np.ascontiguousarray(W.T)
    pet = np.ascontiguousarray(pe.T)
    ones = np.zeros((P, 4), dtype=np.float32)
    ones[:, 0] = 1.0
    in_maps = [
        {
            "tok": np.ascontiguousarray(token_ids[b]).astype(np.int32),
            "wt": wt,
            "pet": pet,
            "pes": pe,
            "onescol": ones,
        }
        for b in range(B)
    ]
    res = run_bass_kernel_spmd(nc, in_maps, list(range(B)), **spmd_kwargs)
    full = np.stack([res.results[b]["out"] for b in range(B)], axis=0)
    return full.astype(np.float32), res


def kernel(token_ids, W_bil):
    full, _ = _run(token_ids, W_bil)
    return full


# revision 26
# speedup vs baseline: 1.0353x; 1.0353x over previous
"""Trainium2 Bass kernel for nn_AttnOnlyTransformer (batch 8, S=D=V=2048).

Sharding: data-parallel over batch — core b computes batch b end-to-end.
No collectives.

Math (per batch), with enc = one_hot(tok) + PE  [S, D]:
  encWt = W @ enc^T                          [D, S]   (stage 1)
  simsT = encWt^T @ enc^T ... = (enc W^T enc^T)^T / sqrt(D) in [k, q] layout
  eT    = exp(simsT * scale) * causal(k<=q)  [S, S]   (stage 2, unnormalized)
  out   = (eT^T @ enc_ext) row-normalized    [S, D]   (stage 3)
where enc_ext = [enc | 1 0 0 0]; the ones column accumulates the softmax
denominator Z in the same pass, and out = numerator * (1/Z).  Softmax without
max-subtraction is exact here: |sims/sqrt(D)| <= ~3, exp can't overflow.

All matmuls run in float32r (full PE rate at N>=512, ~1.5e-4 rel error).
Intermediates encWt and eT are staged through DRAM to fit SBUF.
"""

import numpy as np

import concourse.bass as bass  # noqa: F401  (engine types referenced via nc)
import concourse.mybir as mybir
import concourse.tile as tile
from concourse import bacc
from concourse.bass_utils import run_bass_kernel_spmd
from concourse.masks import make_upper_triangular

P = 128
S = 2048
D = 2048
T = S // P          # 16 tiles per dim
CH = 512            # matmul moving-dim chunk
NCH = S // CH       # 4 chunks
B = 8
f32 = mybir.dt.float32
f32r = mybir.dt.float32r
i32 = mybir.dt.int32
SCALE = 1.0 / float(np.sqrt(np.float32(D)))
NEG = -1.0e9


def _build():
    nc = bacc.Bacc(None, target_bir_lowering=False)
    tok = nc.dram_tensor("tok", [S], i32, kind="ExternalInput")
    wt = nc.dram_tensor("wt", [D, D], f32r, kind="ExternalInput")   # wt[d,e] = W[e,d]
    pet = nc.dram_tensor("pet", [D, S], f32r, kind="ExternalInput")  # PE^T
    pes = nc.dram_tensor("pes", [S, D], f32r, kind="ExternalInput")  # PE
    onescol = nc.dram_tensor("onescol", [P, 4], f32r, kind="ExternalInput")
    out = nc.dram_tensor("out", [S, D], f32, kind="ExternalOutput")

    pet3 = pet.rearrange("(dt p) s -> dt p s", p=P)
    pes3 = pes.rearrange("(st p) d -> st p d", p=P)
    tok2 = tok.rearrange("(st p) -> st p", p=P)
    out3 = out.rearrange("(qt p) d -> qt p d", p=P)

    with tile.TileContext(nc) as tc:
        with (
            tc.tile_pool(name="persist", bufs=1) as persist,
            tc.tile_pool(name="dram", bufs=1, space="DRAM") as dpool,
        ):
            encwt_d = dpool.tile([T, D, P], f32r)     # [kt][e][k_local]
            et_d = dpool.tile([T, T, P, P], f32r)     # [kt][qt][k][q]

            # --- persistent small tiles ---
            iota_col_i = persist.tile([P, 1], i32)
            nc.gpsimd.iota(iota_col_i[:], [[0, 1]], base=0, channel_multiplier=1)
            iota_col_f = persist.tile([P, 1], f32)
            nc.vector.tensor_copy(iota_col_f[:], iota_col_i[:])
            iota_free_i = persist.tile([P, S], i32)
            nc.gpsimd.iota(iota_free_i[:], [[1, S]], base=0, channel_multiplier=0)
            iota_free_f = persist.tile([P, S], f32)
            nc.vector.tensor_copy(iota_free_f[:], iota_free_i[:])
            # additive causal mask for the diagonal block, [k_local, q_local]:
            # 0 where k <= q (keep), NEG where k > q
            maskneg = persist.tile([P, P], f32)
            nc.gpsimd.memset(maskneg[:], 0.0)
            nc.gpsimd.affine_select(
                out=maskneg[:],
                in_=maskneg[:],
                pattern=[[1, P]],
                compare_op=mybir.AluOpType.is_ge,
                fill=NEG,
                base=0,
                channel_multiplier=-1,
            )
            dcols = []
            for dt in range(T):
                dc = persist.tile([P, 1], f32, tag=f"dcol{dt}")
                nc.vector.tensor_scalar_add(dc[:], iota_col_f[:], float(dt * P))
                dcols.append(dc)
            toksb_i = persist.tile([P, T], i32)
            nc.scalar.dma_start(toksb_i[:], tok2.rearrange("st p -> p st"))
            toksb_f = persist.tile([P, T], f32)
            nc.vector.tensor_copy(toksb_f[:], toksb_i[:])
            tokcols = [toksb_f[:, st:st + 1] for st in range(T)]
            tokrow_i = persist.tile([1, S], i32)
            nc.scalar.dma_start(tokrow_i[:], tok[None, :])

            # ================= phase A: encT, stage 1, stage 2 =================
            with (
                tc.tile_pool(name="tokbc", bufs=1) as tokbcp,
                tc.tile_pool(name="enct", bufs=1) as enctp,
                tc.tile_pool(name="wew", bufs=2) as wew,
                tc.tile_pool(name="stgA", bufs=4) as stgA,
                tc.tile_pool(name="psA", bufs=2, space="PSUM") as psA,
            ):
                tok_bc_i = tokbcp.tile([P, S], i32)
                nc.gpsimd.partition_broadcast(tok_bc_i[:], tokrow_i[:])
                tok_bc_f = tokbcp.tile([P, S], f32)
                nc.vector.tensor_copy(tok_bc_f[:], tok_bc_i[:])

                # stage 0: encT[dt][d_local, s] = PE^T[d, s] + (d == tok_s)
                encT = []
                for dt in range(T):
                    e = enctp.tile([P, S], f32r, tag=f"encT{dt}")
                    nc.sync.dma_start(e[:], pet3[dt])
                    nc.vector.scalar_tensor_tensor(
                        e[:],
                        tok_bc_f[:],
                        dcols[dt][:],
                        e[:].bitcast(f32),
                        mybir.AluOpType.is_equal,
                        mybir.AluOpType.add,
                    )
                    encT.append(e)

                # stage 1: encWt[e, k] = sum_d wt[d, e] * encT[d, k] -> DRAM kt-blocked
                # The first 6 psum groups are emitted dt-outer so the PE has
                # several independent accumulations to interleave while the
                # encT tiles are still streaming in from HBM.
                def s1_store(et, kc, ps):
                    o = stgA.tile([P, CH], f32r, tag="s1o")
                    nc.vector.tensor_copy(o[:], ps[:])
                    nc.sync.dma_start(
                        encwt_d[4 * kc:4 * kc + 4, et * P:(et + 1) * P, :]
                        .rearrange("kt p k -> p kt k"),
                        o[:].rearrange("p (kt k) -> p kt k", k=P),
                    )

                def load_w(et):
                    w_sb = wew.tile([P, T, P], f32r, tag="wew")
                    nc.scalar.dma_start(
                        w_sb[:],
                        wt[:, et * P:(et + 1) * P].rearrange("(dt p) e -> p dt e", p=P),
                    )
                    return w_sb

                w0 = load_w(0)
                w1 = load_w(1)
                head = [(0, kc) for kc in range(NCH)] + [(1, 0), (1, 1)]
                head_ps = {g: psA.tile([P, CH], f32, tag="ps1", bufs=6, name=f"hps{g[0]}_{g[1]}") for g in head}
                for dt in range(T):
                    for (et, kc) in head:
                        w_sb = w0 if et == 0 else w1
                        nc.tensor.matmul(
                            head_ps[(et, kc)][:],
                            w_sb[:, dt],
                            encT[dt][:, kc * CH:(kc + 1) * CH],
                            start=(dt == 0),
                            stop=(dt == T - 1),
                        )
                for (et, kc) in head:
                    s1_store(et, kc, head_ps[(et, kc)])

                for et in range(1, T):
                    w_sb = w1 if et == 1 else load_w(et)
                    for kc in range(2 if et == 1 else 0, NCH):
                        ps = psA.tile([P, CH], f32, tag="ps1", bufs=6)
                        for dt in range(T):
                            nc.tensor.matmul(
                                ps[:],
                                w_sb[:, dt],
                                encT[dt][:, kc * CH:(kc + 1) * CH],
                                start=(dt == 0),
                                stop=(dt == T - 1),
                            )
                        s1_store(et, kc, ps)

                # stage 2: eT[k, q] = exp(scale * sum_e encWt[e,k] encT[e,q]) * causal
                for kt in range(T):
                    ew = wew.tile([P, T, P], f32r, tag="wew")
                    nc.scalar.dma_start(
                        ew[:], encwt_d[kt].rearrange("(et p) k -> p et k", p=P)
                    )
                    qc0 = kt * P // CH
                    for qc in range(qc0, NCH):
                        ps = psA.tile([P, CH], f32, tag="ps2")
                        for et in range(T):
                            nc.tensor.matmul(
                                ps[:],
                                ew[:, et],
                                encT[et][:, qc * CH:(qc + 1) * CH],
                                start=(et == 0),
                                stop=(et == T - 1),
                            )
                        if qc == qc0:
                            off = kt * P - qc0 * CH
                            nc.vector.tensor_tensor(
                                ps[:, off:off + P],
                                ps[:, off:off + P],
                                maskneg[:],
                                mybir.AluOpType.add,
                            )
                        eo = stgA.tile([P, CH], f32r, tag="s2o")
                        nc.scalar.activation(
                            eo[:], ps[:], mybir.ActivationFunctionType.Exp, scale=SCALE
                        )
                        nc.sync.dma_start(
                            et_d[kt, 4 * qc:4 * qc + 4].rearrange("qt k q -> k qt q"),
                            eo[:].rearrange("k (qt q) -> k qt q", q=P),
                        )

            # ================= phase B: enc_ext, stage 3 =================
            with (
                tc.tile_pool(name="enc", bufs=1) as encp,
                tc.tile_pool(name="et3", bufs=2) as etp,
                tc.tile_pool(name="stgB", bufs=4) as stgB,
                tc.tile_pool(name="psB", bufs=2, space="PSUM") as psB,
            ):
                # enc_ext[st][s_local, 0:D] = PE[s, d] + (d == tok_s); [:, D:D+4] = 1,0,0,0
                enc = []

                def build_enc(st):
                    e = encp.tile([P, D + 4], f32r, tag=f"enc{st}", name=f"enc{st}")
                    eng = nc.sync if st % 2 == 0 else nc.scalar
                    eng.dma_start(e[:, 0:D], pes3[st])
                    nc.vector.scalar_tensor_tensor(
                        e[:, 0:D],
                        iota_free_f[:],
                        tokcols[st][:],
                        e[:, 0:D].bitcast(f32),
                        mybir.AluOpType.is_equal,
                        mybir.AluOpType.add,
                    )
                    nc.scalar.dma_start(e[:, D:D + 4], onescol[:])
                    enc.append(e)

                for st in range(T):
                    build_enc(st)

                # stage 3: per q-tile, accumulate numerator and Z over k-tiles
                for qt in range(T):
                    ets = []
                    for kt in range(qt + 1):
                        etile = etp.tile([P, P], f32r, tag=f"et{kt}", name=f"et{kt}", bufs=3)
                        nc.scalar.dma_start(etile[:], et_d[kt, qt])
                        ets.append(etile)
                    pss = []
                    for dc in range(NCH):
                        ps = psB.tile([P, CH], f32, tag="ps3d", bufs=6, name=f"ps3d{dc}")
                        for kt in range(qt + 1):
                            nc.tensor.matmul(
                                ps[:],
                                ets[kt][:],
                                enc[kt][:, dc * CH:(dc + 1) * CH],
                                start=(kt == 0),
                                stop=(kt == qt),
                            )
                        pss.append(ps)
                    zps = psB.tile([P, 4], f32, tag="ps3z")
                    for kt in range(qt + 1):
                        nc.tensor.matmul(
                            zps[:],
                            ets[kt][:],
                            enc[kt][:, D:D + 4],
                            start=(kt == 0),
                            stop=(kt == qt),
                        )
                    rz = stgB.tile([P, 1], f32, tag="rz")
                    nc.vector.reciprocal(rz[:], zps[:, 0:1])
                    for dc in range(NCH):
                        ob = stgB.tile([P, CH], f32, tag="ob")
                        nc.scalar.mul(ob[:], pss[dc][:], rz[:])
                        nc.sync.dma_start(out3[qt, :, dc * CH:(dc + 1) * CH], ob[:])

    nc.finalize()
    return nc


def _sinusoidal_pe(seq_len, d_model):
    pos = np.arange(seq_len, dtype=np.float32)[:, None]
    div = np.exp(
        np.arange(0, d_model, 2, dtype=np.float32) * (-np.log(10000.0) / d_model)
    ).astype(np.float32)
    ang = pos * div
    pe = np.zeros((seq_len, d_model), dtype=np.float32)
    pe[:, 0::2] = np.sin(ang)
    pe[:, 1::2] = np.cos(ang)
    return pe


_CACHED_NC = None


def _run(token_ids, W_bil, **spmd_kwargs):
    global _CACHED_NC
    if _CACHED_NC is None:
        _CACHED_NC = _build()
    nc = _CACHED_NC

    token_ids = np.asarray(token_ids)
    W = np.asarray(W_bil, dtype=np.float32)
    assert token_ids.shape == (B, S) and W.shape == (D, D)

    pe = _sinusoidal_pe(S, D)
    wt = np.ascontiguousarray(W.T)
    pet = np.ascontiguousarray(pe.T)
    ones = np.zeros((P, 4), dtype=np.float32)
    ones[:, 0] = 1.0
    in_maps = [
        {
            "tok": np.ascontiguousarray(token_ids[b]).astype(np.int32),
            "wt": wt,
            "pet": pet,
            "pes": pe,
            "onescol": ones,
        }
        for b in range(B)
    ]
    res = run_bass_kernel_spmd(nc, in_maps, list(range(B)), **spmd_kwargs)
    full = np.stack([res.results[b]["out"] for b in range(B)], axis=0)
    return full.astype(np.float32), res


def kernel(token_ids, W_bil):
    full, _ = _run(token_ids, W_bil)
    return full
